# revision 15
# baseline (speedup 1.0000x reference)
"""Trainium2 Bass kernel for nn_NodeEdge (gnn_message_passing).

Computes out[b] = (w * inci + b) @ x[b] : [N,E] x [B,E,F] -> [B,N,F]
with N=4096, E=8192, F=256, B=16 (all fp32).

Strategy (8 NeuronCores):
  - Shard the CONTRACTION dim E across the 8 cores (1024 edges each).
    Each core writes a bf16 partial output [B, F, N]; the host sums the
    8 partials in fp32 and transposes to [B, N, F].
  - MIXED PRECISION contraction: of each core's 8 e-tiles (128 each),
    tiles 0-1 run in fp8e4 (e4m3) with MatmulPerfMode.DoubleRow (2x PE
    rate: one instruction contracts 256 e over a 256-col n-span), tiles
    2-7 run in bf16.  alpha=0.25 of the contraction in fp8 keeps the
    quantization error at ~1.9e-2 rel Frobenius (gate 2e-2) and cuts
    PE time by alpha/2 = 12.5%.
  - To let fp8 and bf16 matmuls accumulate in the SAME PSUM bank, all
    m operands are pre-scaled by 2^9 and all x operands by 2^4 (host
    side, exact powers of two); the drain is a scalar-engine ACTIVATE
    Copy with scale 2^-13 (free).  e4m3 max 240 > scaled maxima (~98,
    ~87), and the scaling lifts m (sigma 0.01) out of the fp8 subnormal
    range.
  - All heavy inputs are host-packed PARTITION-MAJOR; w/b/inci ship as
    a single interleaved byte tensor per node block (one DMA emit).
    VectorE builds mT in matmul-rhs layout [e, n]: bf16 tiles as
    mul+add, fp8 tiles as mul -> bf16 tmp, add -> fp8 out (direct DVE
    fp8 cast).  x ships as bf16 (tiles 2-7) + e4m3 (tiles 0-1).
  - Per node-block of 512: 16 batches x 2 f-tiles of groups, each
    group = 1 bf16 matmul (start, zeroes the PSUM bank) + 2 fp8
    DoubleRow n-halves + 5 bf16 matmuls, ScalarE drains with the
    2^-13 scale and f32->bf16 cast, DMA out.
  - Startup: node-block 0 is staged in n-QUARTERS and its first batches
    run as half-width (256-col) accumulation groups, so the PE starts
    as soon as ~2MB of supply has landed; dummy warm-up matmuls ramp
    the PE clock through the HAM window first.
"""

import numpy as np
import ml_dtypes

N, E, F, B = 4096, 8192, 256, 16
NCORES = 8
ESH = E // NCORES      # 1024 contraction elements per core
ET = ESH // 128        # 8 e-tiles per core
ET8 = 2                # e-tiles 0-1 in fp8 DoubleRow
ETB = ET - ET8         # e-tiles 2-7 in bf16
NBLK = 512             # node-block (output columns per psum accumulation)
FT = F // 128          # 2 f-tiles of 128
NJ = N // NBLK         # 8 node blocks
SM = 2.0 ** 9          # m operand scale
SX = 2.0 ** 4          # x operand scale
SOUT = 1.0 / (SM * SX)

_CACHE = {}


def _build_nc():
    import concourse.mybir as mybir
    import concourse.tile as tile
    from concourse import bacc

    f32 = mybir.dt.float32
    bf16 = mybir.dt.bfloat16
    f8 = mybir.dt.float8e4
    u8 = mybir.dt.uint8
    DR = mybir.MatmulPerfMode.DoubleRow

    nc = bacc.Bacc(None, target_bir_lowering=False)
    # Host-packed layouts (partition dim first, one contiguous run per
    # partition per DMA):
    #   x[p, b, et*F+f]  = x[b, (et+2)*128+p, f] * 2^4            (bf16)
    #   x8[p, b*2F+et*F+f] = e4m3(x[b, et*128+p, f] * 2^4), et<2  (fp8)
    #   wbi[p, j-1, :]   = w.T*2^9 | b.T*2^9 | inci.T bytes for node
    #                      block j, each in (et, n512) order      (j=1..7)
    #   wbi0[p, q, :]    = same for node block 0, n-quarter q (et, n128)
    XBB = ETB * F * 2          # bf16 x bytes per batch per partition
    X8B = ET8 * F              # fp8 x bytes per batch per partition
    x_d = nc.dram_tensor("x", [128, B, XBB + X8B], u8, kind="ExternalInput")
    wbi_d = nc.dram_tensor("wbi", [128, NJ - 1, 5 * ET * NBLK], u8, kind="ExternalInput")
    wbi0_d = nc.dram_tensor("wbi0", [128, 4, 5 * ET * 128], u8, kind="ExternalInput")
    o_d = nc.dram_tensor("out", [B, F, N], bf16, kind="ExternalOutput")

    with tile.TileContext(nc) as tc:
        with (
            tc.tile_pool(name="xres", bufs=1) as xpool,
            tc.tile_pool(name="mtp", bufs=2) as mtpool,
            tc.tile_pool(name="stg", bufs=3) as stgpool,
            tc.tile_pool(name="op", bufs=10) as opool,
            tc.tile_pool(name="mm", bufs=7, space="PSUM") as mmpool,
            tc.tile_pool(name="wp", bufs=1, space="PSUM") as warmpool,
        ):
            xgs = [None] * B
            mbs = [None] * NJ   # bf16 m tiles [128, ETB*NBLK]
            m8s = [None] * NJ   # fp8 m tiles  [128, ET8*NBLK]

            def load_x(q):
                # One batch per DMA: finest-grained arrival for the startup
                # bridge (one contiguous 3.5KB run per partition; bf16 bytes
                # then fp8 bytes).
                xt = xpool.tile([128, XBB + X8B], u8, tag=f"x{q}", name=f"x_sb{q}")
                nc.sync.dma_start(out=xt[:], in_=x_d[:, q])
                xgs[q] = xt

            def load_x_bulk(q0, q1):
                # Batches past the startup bridge arrive in bulk: fewer DMA
                # emits (Sync engine) and fewer semaphores (epilogue clears
                # each one individually).
                nb = q1 - q0
                xt = xpool.tile(
                    [128, nb * (XBB + X8B)], u8, tag=f"xb{q0}", name=f"x_sb{q0}_{q1}"
                )
                nc.sync.dma_start(out=xt[:], in_=x_d[:, q0:q1])
                for q in range(q0, q1):
                    xgs[q] = xt[:, (q - q0) * (XBB + X8B) : (q - q0 + 1) * (XBB + X8B)]

            def x_slice(bb, etb, ft):
                c0 = etb * F + ft * 128
                return xgs[bb][:, 0:XBB].bitcast(bf16)[:, c0 : c0 + 128]

            def x8_slice(bb, ft):
                # [128, 2, 128] stationary fp8 weights for DoubleRow
                v = xgs[bb][:, XBB : XBB + X8B].bitcast(f8).rearrange(
                    "p (et f) -> p et f", et=ET8
                )
                return v[:, :, ft * 128 : (ft + 1) * 128]

            def m8_slice(j, h0, h1):
                # [128, 2, h1-h0] moving fp8 rhs for DoubleRow
                v = m8s[j].rearrange("p (et n) -> p et n", n=NBLK)
                return v[:, :, h0:h1]

            def alloc_mt(j):
                mbs[j] = mtpool.tile([128, ETB * NBLK], bf16, tag="mt", name=f"mt{j}")
                m8s[j] = mtpool.tile([128, ET8 * NBLK], f8, tag="m8", name=f"m8_{j}")

            def prep_full(j):
                # One packed DMA + full-width contiguous VectorE ops.
                alloc_mt(j)
                st = stgpool.tile([128, 5 * 4096], u8, tag="stg", name=f"st{j}")
                nc.sync.dma_start(out=st[:], in_=wbi_d[:, j - 1])
                wv = st[:, 0:8192].bitcast(bf16)
                bv = st[:, 8192:16384].bitcast(bf16)
                iv = st[:, 16384:20480]
                t8 = stgpool.tile([128, ET8 * NBLK], bf16, tag="t8", name=f"t8_{j}")
                nc.vector.tensor_mul(out=t8[:], in0=wv[:, 0:1024], in1=iv[:, 0:1024])
                nc.vector.tensor_add(out=m8s[j][:], in0=t8[:], in1=bv[:, 0:1024])
                nc.vector.tensor_mul(out=mbs[j][:], in0=wv[:, 1024:4096], in1=iv[:, 1024:4096])
                nc.vector.tensor_add(out=mbs[j][:], in0=mbs[j][:], in1=bv[:, 1024:4096])

            def prep0_q(q):
                # Node block 0, n-quarter q: small packed DMA so m[0]
                # becomes usable piecewise as bytes land.
                st = stgpool.tile([128, 5 * 1024], u8, tag="stg0", name=f"st0_{q}")
                nc.sync.dma_start(out=st[:], in_=wbi0_d[:, q])
                wv = st[:, 0:2048].bitcast(bf16).rearrange("p (et n) -> p et n", n=128)
                bv = st[:, 2048:4096].bitcast(bf16).rearrange("p (et n) -> p et n", n=128)
                iv = st[:, 4096:5120].rearrange("p (et n) -> p et n", n=128)
                t0 = stgpool.tile([128, ET8 * 128], bf16, tag="t0", name=f"t0_{q}")
                t03 = t0.rearrange("p (et n) -> p et n", n=128)
                dst8 = m8s[0].rearrange("p (et n) -> p et n", n=NBLK)[
                    :, :, q * 128 : (q + 1) * 128
                ]
                nc.vector.tensor_mul(out=t03, in0=wv[:, 0:ET8], in1=iv[:, 0:ET8])
                nc.vector.tensor_add(out=dst8, in0=t03, in1=bv[:, 0:ET8])
                dstb = mbs[0].rearrange("p (et n) -> p et n", n=NBLK)[
                    :, :, q * 128 : (q + 1) * 128
                ]
                nc.vector.tensor_mul(out=dstb, in0=wv[:, ET8:ET], in1=iv[:, ET8:ET])
                nc.vector.tensor_add(out=dstb, in0=dstb, in1=bv[:, ET8:ET])

            def group(j, bb, ft, n0, n1, tail_split=False):
                ps = mmpool.tile([128, NBLK], f32, tag="ps", name=f"ps{j}_{bb}_{ft}_{n0}")
                # bf16 tile 0 starts the group (start=True zeroes the bank);
                # the fp8 DoubleRow n-spans are interleaved between bf16
                # matmuls so every 136ns fp8 LDWEIGHTS hides under a >=107ns
                # predecessor matmul (two back-to-back DoubleRows would stall
                # ~29ns on the second weight load).
                # One DoubleRow can span the full 512 cols (rhs free 1024):
                # verified correct + fastest on hw (no second weight-load
                # stall, one fewer instruction).
                dr_spans = [(n0, n1)]

                def emit_dr(h, he):
                    nc.tensor.matmul(
                        ps[:, h:he],
                        lhsT=x8_slice(bb, ft),
                        rhs=m8_slice(j, h, he),
                        start=False,
                        stop=False,
                        perf_mode=DR,
                    )

                nc.tensor.matmul(
                    ps[:, n0:n1],
                    lhsT=x_slice(bb, 0, ft),
                    rhs=mbs[j][:, n0:n1],
                    start=True,
                    stop=False,
                )
                for h, he in dr_spans:
                    emit_dr(h, he)
                for etb in range(1, ETB):
                    nc.tensor.matmul(
                        ps[:, n0:n1],
                        lhsT=x_slice(bb, etb, ft),
                        rhs=mbs[j][:, etb * NBLK + n0 : etb * NBLK + n1],
                        start=False,
                        stop=(etb == ETB - 1),
                    )
                ot = opool.tile([128, NBLK], bf16, tag="o", name=f"o{j}_{bb}_{ft}_{n0}")
                orow = o_d[bb, ft * 128 : (ft + 1) * 128]
                if tail_split:
                    # Pipeline the final drain: half-drain, half-DMA-out, so
                    # the kernel tail is half a drain shorter.
                    mid = (n0 + n1) // 2
                    nc.scalar.mul(ot[:, n0:mid], ps[:, n0:mid], SOUT)
                    nc.sync.dma_start(
                        out=orow[:, j * NBLK + n0 : j * NBLK + mid], in_=ot[:, n0:mid]
                    )
                    nc.scalar.mul(ot[:, mid:n1], ps[:, mid:n1], SOUT)
                    nc.sync.dma_start(
                        out=orow[:, j * NBLK + mid : j * NBLK + n1], in_=ot[:, mid:n1]
                    )
                else:
                    nc.scalar.mul(ot[:, n0:n1], ps[:, n0:n1], SOUT)
                    nc.sync.dma_start(
                        out=orow[:, j * NBLK + n0 : j * NBLK + n1], in_=ot[:, n0:n1]
                    )

            def mms(j, b_lo=0, b_hi=B):
                for bb in range(b_lo, b_hi):
                    for ft in range(FT):
                        last = j == NJ - 1 and bb == B - 1 and ft == FT - 1
                        group(j, bb, ft, 0, NBLK, tail_split=last)

            # ---- software pipeline ----
            # PE pre-warm first (no input deps): dummy matmuls on a zeroed
            # tile keep the PE busy through the HAM activity window while the
            # first real supply streams in, so the real matmuls start at
            # K=8/8 (2.4 GHz) instead of paying the 1.2 GHz cold ramp.
            wz = stgpool.tile([128, NBLK], bf16, tag="warm", name="wz")
            nc.vector.memset(wz[:], 0.0)
            ps_warm = warmpool.tile([128, NBLK], f32, tag="psw", name="ps_warm")
            for _ in range(30):
                nc.tensor.matmul(
                    ps_warm[:], lhsT=wz[:, 0:128], rhs=wz[:], start=True, stop=True
                )
            load_x(0)
            alloc_mt(0)
            prep0_q(0)
            prep0_q(1)
            load_x(1)
            load_x(2)
            load_x(3)
            prep0_q(2)
            prep0_q(3)
            prep_full(1)
            load_x_bulk(4, 10)
            load_x_bulk(10, B)
            # Node block 0: first batches run half-width so matmuls start as
            # soon as x batches 0-1 + wbi0 quarters 0-1 have landed; the
            # n-halves are swept separately so the second half only needs
            # quarters 2-3.
            for bb in range(4):
                for ft in range(FT):
                    group(0, bb, ft, 0, 256)
            for bb in range(4):
                for ft in range(FT):
                    group(0, bb, ft, 256, NBLK)
            for bb in range(4, B):
                for ft in range(FT):
                    group(0, bb, ft, 0, NBLK)
            prep_full(2)
            mms(1)
            prep_full(3)
            for j in range(2, NJ):
                mms(j)
                if j + 2 < NJ:
                    prep_full(j + 2)
    nc.finalize()
    return nc


def _get_nc():
    if "nc" not in _CACHE:
        _CACHE["nc"] = _build_nc()
    return _CACHE["nc"]


def run(inputs, trace=False, tmpdir=None, trace_cores=None):
    """Shard + host-pack inputs, run the SPMD bass kernel on 8 cores,
    return (full_output, BassKernelResults)."""
    from concourse.bass_utils import run_bass_kernel_spmd

    bf16 = ml_dtypes.bfloat16
    f8 = ml_dtypes.float8_e4m3
    x = np.asarray(inputs["x"], dtype=np.float32)
    w = np.asarray(inputs["w"], dtype=np.float32)
    inci = np.asarray(inputs["inci"], dtype=np.float32)
    b = np.asarray(inputs["b"], dtype=np.float32)
    assert x.shape == (B, E, F) and w.shape == (N, E)

    in_maps = []
    for c in range(NCORES):
        sl = slice(c * ESH, (c + 1) * ESH)
        wT = w[:, sl].T * SM  # [ESH, N], pre-scaled
        bT = b[:, sl].T * SM
        iT = inci[:, sl].T

        def block(a, j0, j1, nsub, dt):
            # [ESH, ncols] -> [128, nblocks, et-major bytes]
            t = a[:, j0:j1]
            nb = (j1 - j0) // nsub
            t = t.reshape(ET, 128, nb, nsub).transpose(1, 2, 0, 3)  # [128, nb, et, nsub]
            t = np.ascontiguousarray(t.astype(dt))
            return t.reshape(128, nb, -1).view(np.uint8)

        # node blocks 1..7: w|b|i packed per block, (et, n512) order
        wbi = np.concatenate(
            [
                block(wT, NBLK, N, NBLK, bf16),
                block(bT, NBLK, N, NBLK, bf16),
                block(iT, NBLK, N, NBLK, np.uint8),
            ],
            axis=2,
        )
        # node block 0 in quarters, (et, n128) order
        wbi0 = np.concatenate(
            [
                block(wT, 0, NBLK, 128, bf16),
                block(bT, 0, NBLK, 128, bf16),
                block(iT, 0, NBLK, 128, np.uint8),
            ],
            axis=2,
        )
        # x per batch: bf16 bytes (e-tiles 2..7) then fp8 bytes (e-tiles 0..1)
        xsc = x[:, sl, :] * SX  # [B, 1024, F]
        xp = np.ascontiguousarray(
            xsc[:, ET8 * 128 :, :].reshape(B, ETB, 128, F).transpose(2, 0, 1, 3).astype(bf16)
        ).reshape(128, B, ETB * F).view(np.uint8)
        x8p = np.ascontiguousarray(
            xsc[:, : ET8 * 128, :].reshape(B, ET8, 128, F).transpose(2, 0, 1, 3).astype(f8)
        ).reshape(128, B, ET8 * F).view(np.uint8)
        xcomb = np.concatenate([xp, x8p], axis=2)
        in_maps.append({"x": xcomb, "wbi": wbi, "wbi0": wbi0})

    nc = _get_nc()
    res = run_bass_kernel_spmd(
        nc,
        in_maps,
        core_ids=list(range(NCORES)),
        trace=trace,
        tmpdir=tmpdir,
        trace_cores=trace_cores,
    )
    # Sum the 8 bf16 partial products in fp32 and transpose [B,F,N]->[B,N,F].
    total = res.results[0]["out"].astype(np.float32)
    for c in range(1, NCORES):
        total = total + res.results[c]["out"].astype(np.float32)
    out = np.ascontiguousarray(total.transpose(0, 2, 1))
    return out, res


def kernel(x, inci, w, b):
    out, _ = run({"x": x, "inci": inci, "w": w, "b": b})
    return out
